# revision 6
# baseline (speedup 1.0000x reference)
"""MoE layer (8 experts, top-2) on 8 TRN2 NeuronCores — expert-parallel.

Strategy (matches the sharding hint's expert-parallel scheme, minus the
dynamic all-to-all): core e owns expert e's weights. Every core computes
the router in fp32 for all 4096 tokens, derives its own expert's gate
column (renormalized top-2 softmax weight, which reduces to
sigmoid(l_top1 - l_top2)), runs the dense SwiGLU FFN in bf16 for all
tokens in transposed layout (xT [D,T] -> hT [F,T] -> yT [D,T]), scales
by the gate, and a ReduceScatter(add) sums the 8 expert contributions,
leaving each core with a 128-row shard of yT. The host concatenates the
shards and transposes back. Tokens not routed to an expert have gate 0,
so the dense compute is exact.
"""

import os
import sys

sys.path.insert(0, "/opt/trn_rl_repo")

from contextlib import ExitStack

import numpy as np
import ml_dtypes

from concourse import bacc, bass, mybir, tile
from concourse.bass_utils import run_bass_kernel_spmd

P = 128
D = 1024
F = 4096
E = 8
T = int(os.environ.get("MOE_T", "4096"))
NCORES = 8
TCH = min(512, T)  # token chunk held in SBUF as hT
NSUB = min(512, T)  # matmul moving free dim / PSUM bank width

f32 = mybir.dt.float32
bf16 = mybir.dt.bfloat16
AF = mybir.ActivationFunctionType
ALU = mybir.AluOpType

_CACHE = {}


def _body(ctx, tc):
    nc = tc.nc

    xT_bf = nc.dram_tensor("xt_bf", [D, T], bf16, kind="ExternalInput").ap()
    xT_f32 = nc.dram_tensor("xt_f32", [D + 8, T], f32, kind="ExternalInput").ap()
    rwT = nc.dram_tensor("rwt", [D + 8, E], f32, kind="ExternalInput").ap()
    esel = nc.dram_tensor("esel", [P, E], f32, kind="ExternalInput").ap()
    w1s = nc.dram_tensor("w1s", [D, F], bf16, kind="ExternalInput").ap()
    w3s = nc.dram_tensor("w3s", [D, F], bf16, kind="ExternalInput").ap()
    w2s = nc.dram_tensor("w2s", [F, D], bf16, kind="ExternalInput").ap()
    out_sh = nc.dram_tensor("out_shard", [P, T], f32, kind="ExternalOutput").ap()

    consts = ctx.enter_context(tc.tile_pool(name="consts", bufs=1))
    xres = ctx.enter_context(tc.tile_pool(name="xres", bufs=8))
    rtr_x = ctx.enter_context(tc.tile_pool(name="rtr_x", bufs=4))
    rtr_xt = ctx.enter_context(tc.tile_pool(name="rtr_xt", bufs=2))
    rtr_s = ctx.enter_context(tc.tile_pool(name="rtr_s", bufs=6))
    rtr_ps = ctx.enter_context(tc.tile_pool(name="rtr_ps", bufs=2, space="PSUM"))
    wpool = ctx.enter_context(tc.tile_pool(name="wpool", bufs=4))
    w2pool = ctx.enter_context(tc.tile_pool(name="w2pool", bufs=2))
    hpool = ctx.enter_context(tc.tile_pool(name="hpool", bufs=32))
    spool = ctx.enter_context(tc.tile_pool(name="spool", bufs=3))
    ypool = ctx.enter_context(tc.tile_pool(name="ypool", bufs=3))
    psA = ctx.enter_context(tc.tile_pool(name="psA", bufs=2, space="PSUM"))
    psC = ctx.enter_context(tc.tile_pool(name="psC", bufs=2, space="PSUM"))
    dram = ctx.enter_context(tc.tile_pool(name="dram", bufs=1, space="DRAM"))

    gates_hbm = dram.tile([1, T], f32)
    ypart = dram.tile([D, T], f32)
    rs_out = dram.tile([P, T], f32)

    esel_sb = consts.tile([P, E], f32)
    nc.sync.dma_start(esel_sb, esel)
    # router weights: 8 full K-tiles [128, E] + one [8, E] tail (bias row)
    rwt_sb = consts.tile([P, 8, E], f32)
    nc.sync.dma_start(rwt_sb, rwT[0:D, :].rearrange("(k p) e -> p k e", p=P))
    rwb_sb = consts.tile([8, E], f32)
    nc.sync.dma_start(rwb_sb, rwT[D : D + 8, :])

    # ---------------- Router (fp32) ----------------
    for m in range(T // P):
        ps_l = rtr_ps.tile([P, E], f32)
        for k in range(9):
            if k < 8:
                xt = rtr_x.tile([P, P], f32)
                nc.sync.dma_start(xt, xT_f32[k * P : (k + 1) * P, m * P : (m + 1) * P])
                rh = rwt_sb[:, k, :]
            else:
                xt = rtr_xt.tile([8, P], f32)
                nc.sync.dma_start(xt, xT_f32[D : D + 8, m * P : (m + 1) * P])
                rh = rwb_sb[:]
            nc.tensor.matmul(ps_l, lhsT=xt, rhs=rh, start=(k == 0), stop=(k == 8))
        lg = rtr_s.tile([P, E], f32)
        nc.scalar.activation(lg, ps_l, AF.Copy)
        m1 = rtr_s.tile([P, 1], f32)
        nc.vector.tensor_reduce(m1, lg, axis=mybir.AxisListType.X, op=ALU.max)
        # mask out the top-1 slot, then the max of the rest is the top-2 logit
        eqB = rtr_s.tile([P, E], f32)
        nc.vector.tensor_scalar(eqB, lg, m1, 1e30, op0=ALU.is_equal, op1=ALU.mult)
        msk = rtr_s.tile([P, E], f32)
        nc.vector.tensor_tensor(msk, lg, eqB, op=ALU.subtract)
        m2 = rtr_s.tile([P, 1], f32)
        nc.vector.tensor_reduce(m2, msk, axis=mybir.AxisListType.X, op=ALU.max)
        negm2 = rtr_s.tile([P, 1], f32)
        nc.vector.tensor_scalar(negm2, m2, -1.0, None, op0=ALU.mult)
        negm1 = rtr_s.tile([P, 1], f32)
        nc.vector.tensor_scalar(negm1, m1, -1.0, None, op0=ALU.mult)
        # renormalized top-2 softmax weights
        g1 = rtr_s.tile([P, 1], f32)
        nc.scalar.activation(g1, m1, AF.Sigmoid, bias=negm2)
        g2 = rtr_s.tile([P, 1], f32)
        nc.scalar.activation(g2, m2, AF.Sigmoid, bias=negm1)
        eq1g = rtr_s.tile([P, E], f32)
        nc.vector.tensor_scalar(eq1g, lg, m1, g1, op0=ALU.is_equal, op1=ALU.mult)
        eq2g = rtr_s.tile([P, E], f32)
        nc.vector.tensor_scalar(eq2g, lg, m2, g2, op0=ALU.is_equal, op1=ALU.mult)
        ga = rtr_s.tile([P, E], f32)
        nc.vector.tensor_tensor(ga, eq1g, eq2g, op=ALU.add)
        # select this core's expert column via the one-hot esel
        gdump = rtr_s.tile([P, E], f32)
        nc.vector.tensor_tensor(gdump, ga, esel_sb, op=ALU.mult)
        gsel = rtr_s.tile([P, 1], f32)
        nc.vector.tensor_reduce(gsel, gdump, axis=mybir.AxisListType.X, op=ALU.add)
        nc.sync.dma_start(gates_hbm[0:1, m * P : (m + 1) * P], gsel)

    # broadcast gates row to all 128 partitions via PE outer product with ones
    ones_sb = consts.tile([1, P], f32)
    nc.vector.memset(ones_sb, 1.0)
    gates_row = consts.tile([1, T], f32)
    nc.sync.dma_start(gates_row, gates_hbm[0:1, :])
    gates_bc = consts.tile([P, T], f32)
    for c in range(T // NSUB):
        gb = psC.tile([P, NSUB], f32, tag="psy")
        nc.tensor.matmul(
            gb, lhsT=ones_sb[:], rhs=gates_row[0:1, c * NSUB : (c + 1) * NSUB],
            start=True, stop=True,
        )
        nc.scalar.activation(gates_bc[:, c * NSUB : (c + 1) * NSUB], gb, AF.Copy)

    # ---------------- FFN (bf16) ----------------
    xtiles = []
    for k in range(8):
        xt = xres.tile([P, T], bf16)
        nc.sync.dma_start(xt, xT_bf[k * P : (k + 1) * P, :])
        xtiles.append(xt)

    w1r = w1s.rearrange("(k p) f -> p k f", p=P)
    w3r = w3s.rearrange("(k p) f -> p k f", p=P)
    w2r = w2s.rearrange("(k p) d -> p k d", p=P)

    for tci in range(T // TCH):
        t0 = tci * TCH
        htiles = []
        for fm in range(F // P):
            w1t = wpool.tile([P, 8, P], bf16)
            nc.sync.dma_start(w1t, w1r[:, :, fm * P : (fm + 1) * P])
            w3t = wpool.tile([P, 8, P], bf16)
            nc.sync.dma_start(w3t, w3r[:, :, fm * P : (fm + 1) * P])
            ht = hpool.tile([P, TCH], bf16)
            for ns in range(TCH // NSUB):
                n0 = t0 + ns * NSUB
                ps1 = psA.tile([P, NSUB], f32)
                ps3 = psA.tile([P, NSUB], f32)
                for k in range(8):
                    nc.tensor.matmul(
                        ps1, lhsT=w1t[:, k, :], rhs=xtiles[k][:, n0 : n0 + NSUB],
                        start=(k == 0), stop=(k == 7),
                    )
                for k in range(8):
                    nc.tensor.matmul(
                        ps3, lhsT=w3t[:, k, :], rhs=xtiles[k][:, n0 : n0 + NSUB],
                        start=(k == 0), stop=(k == 7),
                    )
                sl = spool.tile([P, NSUB], f32)
                nc.scalar.activation(sl, ps1, AF.Silu)
                nc.vector.tensor_tensor(
                    ht[:, ns * NSUB : (ns + 1) * NSUB], sl, ps3, op=ALU.mult
                )
            htiles.append(ht)
        for dm in range(D // P):
            w2t = w2pool.tile([P, 32, P], bf16)
            nc.sync.dma_start(w2t, w2r[:, :, dm * P : (dm + 1) * P])
            for ns in range(TCH // NSUB):
                n0 = t0 + ns * NSUB
                psy = psC.tile([P, NSUB], f32)
                for k in range(32):
                    nc.tensor.matmul(
                        psy, lhsT=w2t[:, k, :],
                        rhs=htiles[k][:, ns * NSUB : (ns + 1) * NSUB],
                        start=(k == 0), stop=(k == 31),
                    )
                yt = ypool.tile([P, NSUB], f32)
                nc.vector.tensor_tensor(yt, psy, gates_bc[:, n0 : n0 + NSUB], op=ALU.mult)
                nc.sync.dma_start(
                    ypart[dm * P : (dm + 1) * P, n0 : n0 + NSUB], yt
                )

    # ---------------- ReduceScatter + output ----------------
    nc.gpsimd.collective_compute(
        "ReduceScatter",
        ALU.add,
        replica_groups=[list(range(NCORES))],
        ins=[ypart.opt()],
        outs=[rs_out.opt()],
    )
    nc.sync.dma_start(out_sh, rs_out[:])


def _build():
    if "nc" in _CACHE:
        return _CACHE["nc"]
    nc = bacc.Bacc(
        "TRN2",
        target_bir_lowering=False,
        debug=False,
        enable_asserts=False,
        num_devices=NCORES,
    )
    with tile.TileContext(nc) as tc:
        with ExitStack() as ctx:
            _body(ctx, tc)
    nc.compile()
    _CACHE["nc"] = nc
    return nc


def kernel(x, router_w, router_b, w1, w2, w3, _trace=False, _trace_kwargs=None):
    nc = _build()

    xf = np.ascontiguousarray(np.asarray(x, dtype=np.float32).reshape(T, D))
    xT = np.ascontiguousarray(xf.T)  # [D, T]
    aug = np.zeros((8, T), dtype=np.float32)
    aug[0, :] = 1.0  # picks up the router bias row
    xT_f32 = np.ascontiguousarray(np.concatenate([xT, aug], axis=0))
    xT_bf = np.ascontiguousarray(xT.astype(ml_dtypes.bfloat16))

    rw_aug = np.zeros((D + 8, E), dtype=np.float32)
    rw_aug[0:D, :] = np.asarray(router_w, dtype=np.float32).T
    rw_aug[D, :] = np.asarray(router_b, dtype=np.float32)

    in_maps = []
    for c in range(NCORES):
        esel_np = np.zeros((P, E), dtype=np.float32)
        esel_np[:, c] = 1.0
        in_maps.append(
            {
                "xt_bf": xT_bf,
                "xt_f32": xT_f32,
                "rwt": rw_aug,
                "esel": esel_np,
                "w1s": np.ascontiguousarray(w1[c]).astype(ml_dtypes.bfloat16),
                "w3s": np.ascontiguousarray(w3[c]).astype(ml_dtypes.bfloat16),
                "w2s": np.ascontiguousarray(w2[c]).astype(ml_dtypes.bfloat16),
            }
        )

    kw = {}
    if _trace:
        kw["trace"] = True
        kw.update(_trace_kwargs or {})
    res = run_bass_kernel_spmd(nc, in_maps, core_ids=list(range(NCORES)), **kw)
    kernel.last_results = res

    yT = np.concatenate(
        [res.results[c]["out_shard"] for c in range(NCORES)], axis=0
    )  # [D, T]
    return np.ascontiguousarray(yT.T).reshape(np.asarray(x).shape).astype(np.float32)


# revision 25
# speedup vs baseline: 2.9983x; 2.9983x over previous
"""MoE layer (8 experts, top-2) on 8 TRN2 NeuronCores — expert-parallel.

Strategy (matches the sharding hint's expert-parallel scheme, minus the
dynamic all-to-all): core e owns expert e's weights. Every core computes
the router in fp32 for all 4096 tokens, derives its own expert's gate
column (renormalized top-2 softmax weight, which reduces to
sigmoid(l_top1 - l_top2)), runs the dense SwiGLU FFN in bf16 for all
tokens in transposed layout (xT [D,T] -> hT [F,T] -> yT [D,T]), scales
by the gate, and a ReduceScatter(add) sums the 8 expert contributions,
leaving each core with a 128-row shard of yT. The host concatenates the
shards and transposes back. Tokens not routed to an expert have gate 0,
so the dense compute is exact.
"""

import os
import sys

sys.path.insert(0, "/opt/trn_rl_repo")

from contextlib import ExitStack

import numpy as np
import ml_dtypes

from concourse import bacc, bass, mybir, tile
from concourse.bass_utils import run_bass_kernel_spmd

P = 128
D = 1024
F = 4096
E = 8
T = int(os.environ.get("MOE_T", "4096"))
NCORES = 8
TCH = min(512, T)  # token chunk held in SBUF as hT
NSUB = min(512, T)  # matmul moving free dim / PSUM bank width

f32 = mybir.dt.float32
bf16 = mybir.dt.bfloat16
AF = mybir.ActivationFunctionType
ALU = mybir.AluOpType

_CACHE = {}


def _body(ctx, tc):
    nc = tc.nc

    xT_bf = nc.dram_tensor("xt_bf", [D, T], bf16, kind="ExternalInput").ap()
    xT_f32 = nc.dram_tensor("xt_f32", [D + 8, T], f32, kind="ExternalInput").ap()
    rwT = nc.dram_tensor("rwt", [D + 8, E], f32, kind="ExternalInput").ap()
    esel = nc.dram_tensor("esel", [P, E], f32, kind="ExternalInput").ap()
    # weights arrive host-pre-tiled: w1/w3 as [p, fm, k, m], w2 as [p, dm, k, m]
    # so each per-(fm|dm) strip DMA reads one contiguous 2-8 KiB run per partition
    w1s = nc.dram_tensor("w1s", [P, F // P, D // P, P], bf16, kind="ExternalInput").ap()
    w3s = nc.dram_tensor("w3s", [P, F // P, D // P, P], bf16, kind="ExternalInput").ap()
    w2s = nc.dram_tensor("w2s", [P, D // P, F // P, P], bf16, kind="ExternalInput").ap()
    out_sh = nc.dram_tensor("out_shard", [P, T], f32, kind="ExternalOutput").ap()

    consts = ctx.enter_context(tc.tile_pool(name="consts", bufs=1))
    xres = ctx.enter_context(tc.tile_pool(name="xres", bufs=8))
    rtr_x = ctx.enter_context(tc.tile_pool(name="rtr_x", bufs=8))
    rtr_xt = ctx.enter_context(tc.tile_pool(name="rtr_xt", bufs=2))
    rtr_b = ctx.enter_context(tc.tile_pool(name="rtr_b", bufs=1))
    grow_p = ctx.enter_context(tc.tile_pool(name="grow_p", bufs=2))
    rtr_ps = ctx.enter_context(tc.tile_pool(name="rtr_ps", bufs=2, space="PSUM"))
    wpool = ctx.enter_context(tc.tile_pool(name="wpool", bufs=5))
    w2pool = ctx.enter_context(tc.tile_pool(name="w2pool", bufs=3))
    hpool = ctx.enter_context(tc.tile_pool(name="hpool", bufs=32))
    spool = ctx.enter_context(tc.tile_pool(name="spool", bufs=3))
    ypool = ctx.enter_context(tc.tile_pool(name="ypool", bufs=3))
    psA = ctx.enter_context(tc.tile_pool(name="psA", bufs=2, space="PSUM"))
    psC = ctx.enter_context(tc.tile_pool(name="psC", bufs=2, space="PSUM"))
    dram = ctx.enter_context(tc.tile_pool(name="dram", bufs=1, space="DRAM"))

    gates_hbm = dram.tile([1, T], f32)
    NCH = T // TCH
    yparts = [
        dram.tile([D, TCH], bf16, tag=f"ypart{i}", name=f"ypart{i}") for i in range(NCH)
    ]
    rsouts = [
        dram.tile([P, TCH], bf16, tag=f"rsout{i}", name=f"rsout{i}") for i in range(NCH)
    ]

    esel_sb = consts.tile([P, E], f32)
    nc.sync.dma_start(esel_sb, esel)
    # router weights: 8 full K-tiles [128, E] + one [8, E] tail (bias row)
    rwt_sb = consts.tile([P, 8, E], f32)
    nc.sync.dma_start(rwt_sb, rwT[0:D, :].rearrange("(k p) e -> p k e", p=P))
    rwb_sb = consts.tile([8, E], f32)
    nc.sync.dma_start(rwb_sb, rwT[D : D + 8, :])

    # ---------------- Router (fp32, batched top-2) ----------------
    n_router_tiles = 0 if os.environ.get("MOE_NO_ROUTER") else T // P
    M = n_router_tiles
    if M:
        lgall = rtr_b.tile([P, M, E], f32)
        for m in range(M):
            ps_l = rtr_ps.tile([P, E], f32)
            for k in range(9):
                if k < 8:
                    xt = rtr_x.tile([P, P], f32)
                    nc.sync.dma_start(
                        xt, xT_f32[k * P : (k + 1) * P, m * P : (m + 1) * P]
                    )
                    rh = rwt_sb[:, k, :]
                else:
                    xt = rtr_xt.tile([8, P], f32)
                    nc.sync.dma_start(xt, xT_f32[D : D + 8, m * P : (m + 1) * P])
                    rh = rwb_sb[:]
                nc.tensor.matmul(ps_l, lhsT=xt, rhs=rh, start=(k == 0), stop=(k == 8))
            nc.vector.tensor_copy(out=lgall[:, m, :], in_=ps_l)
        # batched top-2 over the E axis for all M tiles at once
        m1 = rtr_b.tile([P, M], f32)
        nc.vector.tensor_reduce(m1, lgall, axis=mybir.AxisListType.X, op=ALU.max)
        m1b = m1[:, :, None].broadcast_to([P, M, E])
        eq1 = rtr_b.tile([P, M, E], f32)
        nc.vector.tensor_tensor(eq1, lgall, m1b, op=ALU.is_equal)
        eqB = rtr_b.tile([P, M, E], f32)
        nc.vector.tensor_scalar(eqB, eq1, 1e30, None, op0=ALU.mult)
        msk = rtr_b.tile([P, M, E], f32)
        nc.vector.tensor_tensor(msk, lgall, eqB, op=ALU.subtract)
        m2 = rtr_b.tile([P, M], f32)
        nc.vector.tensor_reduce(m2, msk, axis=mybir.AxisListType.X, op=ALU.max)
        dd = rtr_b.tile([P, M], f32)
        nc.vector.tensor_tensor(dd, m1, m2, op=ALU.subtract)
        g1 = rtr_b.tile([P, M], f32)
        nc.scalar.activation(g1, dd, AF.Sigmoid)
        g2 = rtr_b.tile([P, M], f32)
        nc.vector.tensor_scalar(g2, g1, -1.0, 1.0, op0=ALU.mult, op1=ALU.add)
        eq2 = rtr_b.tile([P, M, E], f32)
        nc.vector.tensor_tensor(
            eq2, msk, m2[:, :, None].broadcast_to([P, M, E]), op=ALU.is_equal
        )
        t1 = rtr_b.tile([P, M, E], f32)
        nc.vector.tensor_tensor(
            t1, eq1, g1[:, :, None].broadcast_to([P, M, E]), op=ALU.mult
        )
        t2 = rtr_b.tile([P, M, E], f32)
        nc.vector.tensor_tensor(
            t2, eq2, g2[:, :, None].broadcast_to([P, M, E]), op=ALU.mult
        )
        ga = rtr_b.tile([P, M, E], f32)
        nc.vector.tensor_tensor(ga, t1, t2, op=ALU.add)
        gw = rtr_b.tile([P, M, E], f32)
        nc.vector.tensor_tensor(
            gw, ga, esel_sb[:, None, :].broadcast_to([P, M, E]), op=ALU.mult
        )
        gsel = rtr_b.tile([P, M], f32)
        nc.vector.tensor_reduce(gsel, gw, axis=mybir.AxisListType.X, op=ALU.add)
        for m in range(M):
            nc.sync.dma_start(gates_hbm[0:1, m * P : (m + 1) * P], gsel[:, m : m + 1])

    # broadcast gates row to all 128 partitions via PE outer product with ones
    gates_bc = consts.tile([P, T], f32)
    if n_router_tiles == 0:
        nc.vector.memset(gates_bc, 1.0)
    else:
        ones_sb = consts.tile([1, P], f32)
        nc.vector.memset(ones_sb, 1.0)
        for c in range(T // NSUB):
            grow = grow_p.tile([1, NSUB], f32)
            nc.sync.dma_start(grow, gates_hbm[0:1, c * NSUB : (c + 1) * NSUB])
            gb = psC.tile([P, NSUB], f32, tag="psy")
            nc.tensor.matmul(gb, lhsT=ones_sb[:], rhs=grow[0:1, :], start=True, stop=True)
            nc.scalar.activation(gates_bc[:, c * NSUB : (c + 1) * NSUB], gb, AF.Copy)

    # ---------------- FFN (bf16) ----------------
    xtiles = []
    for k in range(8):
        xt = xres.tile([P, T], bf16)
        for ci in range(T // TCH):
            nc.sync.dma_start(
                xt[:, ci * TCH : (ci + 1) * TCH],
                xT_bf[k * P : (k + 1) * P, ci * TCH : (ci + 1) * TCH],
            )
        xtiles.append(xt)

    for tci in range(T // TCH):
        t0 = tci * TCH
        htiles = []
        for fm in range(F // P):
            w1t = wpool.tile([P, 8, P], bf16)
            nc.sync.dma_start(w1t, w1s[:, fm, :, :])
            w3t = wpool.tile([P, 8, P], bf16)
            nc.sync.dma_start(w3t, w3s[:, fm, :, :])
            ht = hpool.tile([P, TCH], bf16)
            for ns in range(TCH // NSUB):
                n0 = t0 + ns * NSUB
                ps1 = psA.tile([P, NSUB], f32)
                ps3 = psA.tile([P, NSUB], f32)
                for k in range(8):
                    nc.tensor.matmul(
                        ps1, lhsT=w1t[:, k, :], rhs=xtiles[k][:, n0 : n0 + NSUB],
                        start=(k == 0), stop=(k == 7),
                    )
                n_w3 = 1 if os.environ.get("MOE_HALF") else 8
                for k in range(n_w3):
                    nc.tensor.matmul(
                        ps3, lhsT=w3t[:, k, :], rhs=xtiles[k][:, n0 : n0 + NSUB],
                        start=(k == 0), stop=(k == n_w3 - 1),
                    )
                sl = spool.tile([P, NSUB], f32)
                nc.scalar.activation(sl, ps1, AF.Silu)
                nc.vector.tensor_tensor(
                    ht[:, ns * NSUB : (ns + 1) * NSUB], sl, ps3, op=ALU.mult
                )
            htiles.append(ht)
        for dm in range(D // P):
            w2t = w2pool.tile([P, 32, P], bf16)
            nc.sync.dma_start(w2t, w2s[:, dm, :, :])
            for ns in range(TCH // NSUB):
                n0 = t0 + ns * NSUB
                nl = ns * NSUB
                psy = psC.tile([P, NSUB], f32, tag="psy")
                for k in range(32):
                    nc.tensor.matmul(
                        psy, lhsT=w2t[:, k, :],
                        rhs=htiles[k][:, ns * NSUB : (ns + 1) * NSUB],
                        start=(k == 0), stop=(k == 31),
                    )
                yt = ypool.tile([P, NSUB], bf16)
                nc.vector.tensor_tensor(yt, psy, gates_bc[:, n0 : n0 + NSUB], op=ALU.mult)
                nc.sync.dma_start(
                    yparts[tci][dm * P : (dm + 1) * P, nl : nl + NSUB], yt
                )
        # per-chunk ReduceScatter overlaps the next chunk's compute
        if not os.environ.get("MOE_NO_RS"):
            nc.gpsimd.collective_compute(
                "ReduceScatter",
                ALU.add,
                replica_groups=[list(range(NCORES))],
                ins=[yparts[tci].opt()],
                outs=[rsouts[tci].opt()],
            )

    # ---------------- output assembly ----------------
    if os.environ.get("MOE_NO_RS"):
        for tci in range(NCH):
            nc.gpsimd.dma_start(out_sh[:, tci * TCH : (tci + 1) * TCH], yparts[tci][0:P, :])
    else:
        for tci in range(NCH):
            ro = ypool.tile([P, TCH], bf16, tag="rout")
            nc.sync.dma_start(ro, rsouts[tci][:])
            rof = ypool.tile([P, TCH], f32, tag="rof")
            nc.vector.tensor_copy(out=rof, in_=ro)
            nc.sync.dma_start(out_sh[:, tci * TCH : (tci + 1) * TCH], rof)


def _pretile_kpm(w):
    """[K, M] -> [p, mt, kt, m] bf16 so a per-mt strip is one contiguous
    (kt*m) run per partition."""
    K, Mo = w.shape
    kt, mt = K // P, Mo // P
    wt = np.asarray(w, np.float32).reshape(kt, P, mt, P).transpose(1, 2, 0, 3)
    return np.ascontiguousarray(wt.astype(ml_dtypes.bfloat16))


def _build():
    if "nc" in _CACHE:
        return _CACHE["nc"]
    nc = bacc.Bacc(
        "TRN2",
        target_bir_lowering=False,
        debug=False,
        enable_asserts=False,
        num_devices=NCORES,
    )
    with tile.TileContext(nc) as tc:
        with ExitStack() as ctx:
            _body(ctx, tc)
    nc.compile()
    _CACHE["nc"] = nc
    return nc


def kernel(x, router_w, router_b, w1, w2, w3, _trace=False, _trace_kwargs=None):
    nc = _build()

    xf = np.ascontiguousarray(np.asarray(x, dtype=np.float32).reshape(T, D))
    xT = np.ascontiguousarray(xf.T)  # [D, T]
    aug = np.zeros((8, T), dtype=np.float32)
    aug[0, :] = 1.0  # picks up the router bias row
    xT_f32 = np.ascontiguousarray(np.concatenate([xT, aug], axis=0))
    xT_bf = np.ascontiguousarray(xT.astype(ml_dtypes.bfloat16))

    rw_aug = np.zeros((D + 8, E), dtype=np.float32)
    rw_aug[0:D, :] = np.asarray(router_w, dtype=np.float32).T
    rw_aug[D, :] = np.asarray(router_b, dtype=np.float32)

    in_maps = []
    for c in range(NCORES):
        esel_np = np.zeros((P, E), dtype=np.float32)
        esel_np[:, c] = 1.0
        in_maps.append(
            {
                "xt_bf": xT_bf,
                "xt_f32": xT_f32,
                "rwt": rw_aug,
                "esel": esel_np,
                "w1s": _pretile_kpm(w1[c]),
                "w3s": _pretile_kpm(w3[c]),
                "w2s": _pretile_kpm(w2[c]),
            }
        )

    kw = {}
    if _trace:
        kw["trace"] = True
        kw.update(_trace_kwargs or {})
    res = run_bass_kernel_spmd(nc, in_maps, core_ids=list(range(NCORES)), **kw)
    kernel.last_results = res

    yT = np.concatenate(
        [res.results[c]["out_shard"] for c in range(NCORES)], axis=0
    )  # [D, T]
    return np.ascontiguousarray(yT.T).reshape(np.asarray(x).shape).astype(np.float32)
